# revision 20
# baseline (speedup 1.0000x reference)
"""Trainium2 Bass kernel for nn_AttentionLayer (B=128,H=16,L=64,E=128, C=2048).

out[b,l,:] = (softmax(0.1 * q_bh @ k_bh^T) @ v_bh  for h) . W^T + bias

Data-parallel over batch across 8 NeuronCores (16 batches/core, no
collectives).  Per core, 8 two-batch blocks; per block, 16 head-pair
groups processed as 4 "units" of 4 groups:

  - q/k/v and W load via SWDGE (GpSimd) DMAs that cast f32->bf16 inline
    (no staging tiles, no engine cast ops),
  - q/k PE-transposed per group into [e, tok] (bf16, psum) and evacuated
    by DVE into one [e, 2, G, 256] block tile,
  - MM1 per group: scT = k @ q^T (full 128x128; off-diagonal cross-head
    blocks are garbage never read); 4 groups of a unit share one psum
    bank [128, 4, 128],
  - exp: TWO batched scalar-engine calls per unit (diag blocks of all 4
    groups at once) into a pre-zeroed bf16 ring -> expT block-diagonal,
  - MM2 per group with the ring slot as the *stationary* operand and
    [v | 1] moving: U = expT^T @ [v|1] comes out token-major [l, 129]
    with the softmax denominator in column 128,
  - normalize: one [128, 2] DVE reciprocal per U-pair, per-partition
    tensor_scalar multiply -> V2 (bf16, sbuf),
  - V^T: PE transpose pairs, evacuated into the vt tile by the scalar
    engine (ACT is idle after exp batching),
  - projection out = V @ W^T + b: K=2048 accumulated bf16 matmuls,
    emitted as a generator drained between attention ops of the next
    block to keep the PE dense,
  - W-prep (bf16 chunk DMA -> 16 PE transposes -> DVE evac) is also a
    generator, drained through the first blocks as the W stream arrives.

PSUM banks: 2 transposes (q/k, V^T, W-prep), 2 scores (unit parity),
2 U (pair-of-groups tiles), 2 projection.
"""

import numpy as np

import concourse.bass as bass
import concourse.mybir as mybir
import concourse.tile as tile
from concourse import bacc
from concourse.bass_utils import run_bass_kernel_spmd
from concourse.masks import make_identity

N_CORES = 8
ATT_MODE = "bf16"   # kept for test.py compat
MM3_MODE = "bf16"
B, H, L, E = 128, 16, 64, 128
C = H * E                 # 2048
BPC = B // N_CORES        # 16 batches per core
NBLK = BPC // 2           # 8 two-batch blocks per core
G = H // 2                # 8 head-pair groups per batch
NU = 4                    # units per block (4 groups each)
SCALE = 0.1
F32 = mybir.dt.float32
BF16 = mybir.dt.bfloat16


def emit(ctx, nc, tc, q_d, k_d, v_d, w_d, b_d, o_d):
    # DRAM views: [p, b, g, e] where token row (h*64+l) = g*128 + p
    qv = q_d.rearrange("b h l e -> b (h l) e").rearrange("b (g p) e -> p b g e", p=128)
    kv = k_d.rearrange("b h l e -> b (h l) e").rearrange("b (g p) e -> p b g e", p=128)
    vv = v_d.rearrange("b h l e -> b (h l) e").rearrange("b (g p) e -> p b g e", p=128)

    const = ctx.enter_context(tc.tile_pool(name="const", bufs=1))
    wst = ctx.enter_context(tc.tile_pool(name="wst", bufs=3))
    qkvb = ctx.enter_context(tc.tile_pool(name="qkvb", bufs=2))
    qktp = ctx.enter_context(tc.tile_pool(name="qktp", bufs=2))
    v2p = ctx.enter_context(tc.tile_pool(name="v2p", bufs=3))
    vtp = ctx.enter_context(tc.tile_pool(name="vtp", bufs=3))
    outp = ctx.enter_context(tc.tile_pool(name="outp", bufs=3))

    ptr = ctx.enter_context(tc.tile_pool(name="ptr", bufs=2, space="PSUM"))
    pscp = ctx.enter_context(tc.tile_pool(name="pscp", bufs=2, space="PSUM"))
    putp = ctx.enter_context(tc.tile_pool(name="putp", bufs=2, space="PSUM"))
    pmm3 = ctx.enter_context(tc.tile_pool(name="pmm3", bufs=2, space="PSUM"))

    # ---- constants ----
    id_b = const.tile([128, 128], BF16, tag="id_b")
    make_identity(nc, id_b)
    # pre-zeroed exp ring: only diagonal 64x64 blocks are ever (re)written,
    # so off-diagonals stay zero and MM2 contracts all 128 partitions
    exp_ring = const.tile([128, 2 * NU, 128], BF16, tag="expr")
    nc.vector.memset(exp_ring, 0.0)
    bias_bc = const.tile([128, C], BF16, tag="bias")
    wt_sb = const.tile([128, H, C], BF16, tag="wt")

    b_bcast = bass.AP(tensor=b_d.tensor, offset=b_d.offset, ap=[[0, 128]] + list(b_d.ap))
    nc.gpsimd.dma_start(out=bias_bc, in_=b_bcast)

    # ---- block loads: SWDGE DMA with inline f32->bf16 cast ----
    def load_block(m):
        qb = qkvb.tile([128, 2, G, 128], BF16, tag="qb")
        kb = qkvb.tile([128, 2, G, 128], BF16, tag="kb")
        vb = qkvb.tile([128, 2, G, 132], BF16, tag="vb")
        nc.gpsimd.dma_start(out=qb, in_=qv[:, 2 * m : 2 * m + 2, :, :])
        nc.gpsimd.dma_start(out=kb, in_=kv[:, 2 * m : 2 * m + 2, :, :])
        nc.gpsimd.dma_start(out=vb[:, :, :, 0:128], in_=vv[:, 2 * m : 2 * m + 2, :, :])
        nc.vector.memset(vb[:, :, :, 128:129], 1.0)
        return qb, kb, vb

    with nc.named_scope("load0"):
        blk_tiles = load_block(0)

    # ---- W stream: SWDGE DMA-cast chunk -> PE transpose pairs -> DVE evac ----
    wstate = {"chunks": 0}

    def wprep_emitter():
        for nt in range(16):
            wn_c = wst.tile([128, C], BF16, tag="wnc")
            nc.gpsimd.dma_start(out=wn_c, in_=w_d[nt * 128 : (nt + 1) * 128, :])
            yield
            for kk in range(0, 16, 2):
                # transposes as REGULAR matmuls vs identity: they stream at
                # the warm PE clock and count as PE-busy for HAM (the
                # transpose-mode fast path does neither)
                tp = ptr.tile([128, 256], F32, tag="tr", name="tp")
                nc.tensor.matmul(
                    tp[:, 0:128], wn_c[:, kk * 128 : (kk + 1) * 128], id_b,
                    start=True, stop=True,
                )
                yield
                nc.tensor.matmul(
                    tp[:, 128:256], wn_c[:, (kk + 1) * 128 : (kk + 2) * 128], id_b,
                    start=True, stop=True,
                )
                nc.vector.tensor_copy(
                    wt_sb[:, kk : kk + 2, nt * 128 : (nt + 1) * 128],
                    tp.rearrange("p (a b) -> p a b", a=2),
                )
                yield
            wstate["chunks"] = nt + 1

    # ---- projection: out[128 tok, 2048] = V @ W^T + b, K=2048 accumulated ----
    def proj_emitter(m, vt):
        ot = outp.tile([128, C], F32, tag="ot")
        for half in range(2):
            # a proj matmul must not be EMITTED before the wprep writes it
            # reads exist in the program, else Tile cannot order them
            while wstate["chunks"] < 8 * (half + 1) and wq:
                _pull(wq, 17)
            ps = [
                pmm3.tile([128, 512], F32, tag="mm3", name=f"ps{n}")
                for n in range(2)
            ]
            for kk in range(16):
                for n in range(2):
                    nn = half * 2 + n
                    nc.tensor.matmul(
                        ps[n],
                        vt[:, kk, :],
                        wt_sb[:, kk, nn * 512 : (nn + 1) * 512],
                        start=(kk == 0), stop=(kk == 15),
                    )
                    yield
            for n in range(2):
                nn = half * 2 + n
                nc.vector.tensor_add(
                    ot[:, nn * 512 : (nn + 1) * 512],
                    ps[n],
                    bias_bc[:, nn * 512 : (nn + 1) * 512],
                )
                yield
        if m == NBLK - 1:
            for nn in range(4):
                nc.sync.dma_start(
                    out=o_d[m * 128 : (m + 1) * 128, nn * 512 : (nn + 1) * 512],
                    in_=ot[:, nn * 512 : (nn + 1) * 512],
                )
        else:
            nc.sync.dma_start(out=o_d[m * 128 : (m + 1) * 128, :], in_=ot)

    # Two drain queues.  wprep is paced per block (its transposes wait on the
    # W DMA stream — drained too fast they head-block the PE queue); the
    # projection FIFO gets the rest of the budget.
    wq = [wprep_emitter()]
    pq = []
    wbudget = [0]

    def _pull(queue, k):
        while queue and k > 0:
            gen = queue[0]
            try:
                while k > 0:
                    next(gen)
                    k -= 1
            except StopIteration:
                queue.pop(0)
        return k

    def drain(k):
        if wbudget[0] > 0 and wq:
            take = min(k, wbudget[0])
            left = _pull(wq, take)
            wbudget[0] -= take - left
            k = k - (take - left)
        if k > 0:
            _pull(pq, k)

    def drain_all():
        _pull(wq, 1 << 30)
        _pull(pq, 1 << 30)

    # ---- per-(block, unit) software pipeline ----
    # q/k transpose pairs for unit u are emitted during unit u-1 (crossing
    # block seams) so the PE never sees a transpose burst.
    blk_state = {}

    def setup_block(m):
        qb, kb, vb = blk_state[m]["qkv"]
        blk_state[m]["qkt"] = qktp.tile(
            [128, 2, G, 256], BF16, tag="qkt", name=f"qkt{m}"
        )
        blk_state[m]["vt"] = vtp.tile([128, H, 128], BF16, tag="vt", name=f"vt{m}")

    def emit_pairs(m, u):
        # transpose q/k for the 4 groups of unit u of block m
        qb, kb, vb = blk_state[m]["qkv"]
        qkt = blk_state[m]["qkt"]
        bb, g0 = u // 2, (u % 2) * NU
        for j in range(4):
            drain(2)
            g = g0 + j
            trp = ptr.tile([128, 256], F32, tag="tr", name="trp")
            nc.tensor.matmul(
                trp[:, 0:128], qb[:, bb, g, :], id_b, start=True, stop=True
            )
            nc.tensor.matmul(
                trp[:, 128:256], kb[:, bb, g, :], id_b, start=True, stop=True
            )
            nc.vector.tensor_copy(qkt[:, bb, g, :], trp)

    blk_state[0] = {"qkv": blk_tiles}
    setup_block(0)
    emit_pairs(0, 0)

    for m in range(NBLK):
        qb, kb, vb = blk_state[m]["qkv"]
        qkt = blk_state[m]["qkt"]
        vt = blk_state[m]["vt"]
        # wprep pacing: ~4 chunks' worth of yields per early block
        wbudget[0] = 70 if m < 4 else 0
        with nc.named_scope(f"attn{m}"):
            if m + 1 < NBLK:
                with nc.named_scope(f"load{m + 1}"):
                    blk_state[m + 1] = {"qkv": load_block(m + 1)}
            for u in range(NU):
                bb, g0 = u // 2, (u % 2) * NU
                ru = u % 2  # ring half
                # MM1: scores^T for the unit's 4 groups into one psum bank
                sc = pscp.tile([128, 4, 128], F32, tag="sc")
                for j in range(4):
                    drain(2)
                    nc.tensor.matmul(
                        sc[:, j, :],
                        qkt[:, bb, g0 + j, 128:256],
                        qkt[:, bb, g0 + j, 0:128],
                        start=True, stop=True,
                    )
                # batched exp over the 4 groups' diagonal blocks
                for lo, hi in ((0, 64), (64, 128)):
                    nc.scalar.activation(
                        exp_ring[lo:hi, 4 * ru : 4 * ru + 4, lo:hi],
                        sc[lo:hi, :, lo:hi],
                        mybir.ActivationFunctionType.Exp, scale=SCALE,
                    )
                # emit next unit's q/k transposes here so they overlap this
                # unit's exp/MM2 chain
                if u + 1 < NU:
                    emit_pairs(m, u + 1)
                elif m + 1 < NBLK:
                    setup_block(m + 1)
                    emit_pairs(m + 1, 0)
                tok = bb * 64
                for t in range(2):  # pairs of groups
                    drain(2)
                    # MM2: U = expT^T @ [v|1] -> token-major [l, 129] with
                    # the softmax denominator in column 128
                    uu = putp.tile([128, 2, 132], F32, tag="ut")
                    for jj in range(2):
                        j = 2 * t + jj
                        nc.tensor.matmul(
                            uu[:, jj, 0:129],
                            exp_ring[:, 4 * ru + j, :],
                            vb[:, bb, g0 + j, 0:129],
                            start=True, stop=True,
                        )
                        drain(2)
                    r2 = v2p.tile([128, 2], F32, tag="r2")
                    nc.vector.reciprocal(r2, uu[:, :, 128:129])
                    trv = ptr.tile([128, 256], F32, tag="tr", name="trv")
                    for jj in range(2):
                        j = 2 * t + jj
                        v2 = v2p.tile([128, 128], BF16, tag="v2")
                        # per-partition 1/rowsum scale on the scalar engine
                        nc.scalar.mul(v2, uu[:, jj, 0:128], r2[:, jj : jj + 1])
                        nc.tensor.matmul(
                            trv[:, jj * 128 : (jj + 1) * 128], v2, id_b,
                            start=True, stop=True,
                        )
                        drain(1)
                    # evacuate both V^T tiles into vt on the scalar engine
                    h0 = 2 * (g0 + 2 * t)
                    nc.scalar.copy(
                        vt[:, h0 : h0 + 4, tok : tok + 64],
                        trv.rearrange("p (a b) -> p a b", a=4),
                    )
        pq.append(proj_emitter(m, vt))
        if m == NBLK - 1:
            drain_all()


def build():
    import contextlib

    nc = bacc.Bacc("TRN2", target_bir_lowering=False, debug=False)
    q_d = nc.dram_tensor("queries", [BPC, H, L, E], F32, kind="ExternalInput").ap()
    k_d = nc.dram_tensor("keys", [BPC, H, L, E], F32, kind="ExternalInput").ap()
    v_d = nc.dram_tensor("values", [BPC, H, L, E], F32, kind="ExternalInput").ap()
    w_d = nc.dram_tensor("W", [C, C], F32, kind="ExternalInput").ap()
    b_d = nc.dram_tensor("b", [C], F32, kind="ExternalInput").ap()
    o_d = nc.dram_tensor("out", [BPC * L, C], F32, kind="ExternalOutput").ap()

    with tile.TileContext(nc) as tc:
        with contextlib.ExitStack() as ctx:
            emit(ctx, nc, tc, q_d, k_d, v_d, w_d, b_d, o_d)
    nc.compile()
    return nc


_NC_CACHE = {}


def get_nc(*a):
    if "nc" not in _NC_CACHE:
        _NC_CACHE["nc"] = build()
    return _NC_CACHE["nc"]


def make_in_maps(queries, keys, values, W, b):
    queries = np.ascontiguousarray(np.asarray(queries, dtype=np.float32))
    keys = np.ascontiguousarray(np.asarray(keys, dtype=np.float32))
    values = np.ascontiguousarray(np.asarray(values, dtype=np.float32))
    W = np.ascontiguousarray(np.asarray(W, dtype=np.float32))
    b = np.ascontiguousarray(np.asarray(b, dtype=np.float32))
    in_maps = []
    for i in range(N_CORES):
        s = slice(i * BPC, (i + 1) * BPC)
        in_maps.append(
            {
                "queries": queries[s],
                "keys": keys[s],
                "values": values[s],
                "W": W,
                "b": b,
            }
        )
    return in_maps


def kernel(queries, keys, values, W, b, **run_kwargs):
    nc = get_nc()
    in_maps = make_in_maps(queries, keys, values, W, b)
    res = run_bass_kernel_spmd(nc, in_maps, core_ids=list(range(N_CORES)), **run_kwargs)
    out = np.concatenate([res.results[i]["out"] for i in range(N_CORES)], axis=0)
    return out.reshape(B, L, C)


# revision 24
# speedup vs baseline: 1.0493x; 1.0493x over previous
"""Trainium2 Bass kernel for nn_AttentionLayer (B=128,H=16,L=64,E=128, C=2048).

out[b,l,:] = (softmax(0.1 * q_bh @ k_bh^T) @ v_bh  for h) . W^T + bias

Data-parallel over batch across 8 NeuronCores (16 batches/core, no
collectives).  Per core, 8 two-batch blocks; per block, 16 head-pair
groups processed as 4 "units" of 4 groups:

  - q/k/v and W load via SWDGE (GpSimd) DMAs that cast f32->bf16 inline
    (no staging tiles, no engine cast ops),
  - q/k PE-transposed per group into [e, tok] (bf16, psum) and evacuated
    by DVE into one [e, 2, G, 256] block tile,
  - MM1 per group: scT = k @ q^T (full 128x128; off-diagonal cross-head
    blocks are garbage never read); 4 groups of a unit share one psum
    bank [128, 4, 128],
  - exp: TWO batched scalar-engine calls per unit (diag blocks of all 4
    groups at once) into a pre-zeroed bf16 ring -> expT block-diagonal,
  - MM2 per group with the ring slot as the *stationary* operand and
    [v | 1] moving: U = expT^T @ [v|1] comes out token-major [l, 129]
    with the softmax denominator in column 128,
  - normalize: one [128, 2] DVE reciprocal per U-pair, per-partition
    tensor_scalar multiply -> V2 (bf16, sbuf),
  - V^T: PE transpose pairs, evacuated into the vt tile by the scalar
    engine (ACT is idle after exp batching),
  - projection out = V @ W^T + b: K=2048 accumulated bf16 matmuls,
    emitted as a generator drained between attention ops of the next
    block to keep the PE dense,
  - W-prep (bf16 chunk DMA -> 16 PE transposes -> DVE evac) is also a
    generator, drained through the first blocks as the W stream arrives.

PSUM banks: 2 transposes (q/k, V^T, W-prep), 2 scores (unit parity),
2 U (pair-of-groups tiles), 2 projection.
"""

import numpy as np

import concourse.bass as bass
import concourse.mybir as mybir
import concourse.tile as tile
from concourse import bacc
from concourse.bass_utils import run_bass_kernel_spmd
from concourse.masks import make_identity

N_CORES = 8
ATT_MODE = "bf16"   # kept for test.py compat
MM3_MODE = "bf16"
B, H, L, E = 128, 16, 64, 128
C = H * E                 # 2048
BPC = B // N_CORES        # 16 batches per core
NBLK = BPC // 2           # 8 two-batch blocks per core
G = H // 2                # 8 head-pair groups per batch
NU = 4                    # units per block (4 groups each)
SCALE = 0.1
F32 = mybir.dt.float32
BF16 = mybir.dt.bfloat16


def emit(ctx, nc, tc, q_d, k_d, v_d, w_d, b_d, o_d):
    # DRAM views: [p, b, g, e] where token row (h*64+l) = g*128 + p
    qv = q_d.rearrange("b h l e -> b (h l) e").rearrange("b (g p) e -> p b g e", p=128)
    kv = k_d.rearrange("b h l e -> b (h l) e").rearrange("b (g p) e -> p b g e", p=128)
    vv = v_d.rearrange("b h l e -> b (h l) e").rearrange("b (g p) e -> p b g e", p=128)

    const = ctx.enter_context(tc.tile_pool(name="const", bufs=1))
    wst = ctx.enter_context(tc.tile_pool(name="wst", bufs=3))
    qkvb = ctx.enter_context(tc.tile_pool(name="qkvb", bufs=2))
    qktp = ctx.enter_context(tc.tile_pool(name="qktp", bufs=2))
    v2p = ctx.enter_context(tc.tile_pool(name="v2p", bufs=3))
    vtp = ctx.enter_context(tc.tile_pool(name="vtp", bufs=3))
    outp = ctx.enter_context(tc.tile_pool(name="outp", bufs=3))

    ptr = ctx.enter_context(tc.tile_pool(name="ptr", bufs=2, space="PSUM"))
    pscp = ctx.enter_context(tc.tile_pool(name="pscp", bufs=2, space="PSUM"))
    putp = ctx.enter_context(tc.tile_pool(name="putp", bufs=2, space="PSUM"))
    pmm3 = ctx.enter_context(tc.tile_pool(name="pmm3", bufs=2, space="PSUM"))

    # ---- constants ----
    id_b = const.tile([128, 128], BF16, tag="id_b")
    make_identity(nc, id_b)
    # pre-zeroed exp ring: only diagonal 64x64 blocks are ever (re)written,
    # so off-diagonals stay zero and MM2 contracts all 128 partitions
    exp_ring = const.tile([128, 2 * NU, 128], BF16, tag="expr")
    nc.vector.memset(exp_ring, 0.0)
    bias_bc = const.tile([128, C], BF16, tag="bias")
    wt_sb = const.tile([128, H, C], BF16, tag="wt")

    b_bcast = bass.AP(tensor=b_d.tensor, offset=b_d.offset, ap=[[0, 128]] + list(b_d.ap))
    nc.gpsimd.dma_start(out=bias_bc, in_=b_bcast)

    # ---- block loads: SWDGE DMA with inline f32->bf16 cast ----
    def load_block(m):
        qb = qkvb.tile([128, 2, G, 128], BF16, tag="qb")
        kb = qkvb.tile([128, 2, G, 128], BF16, tag="kb")
        vb = qkvb.tile([128, 2, G, 132], BF16, tag="vb")
        nc.gpsimd.dma_start(out=qb, in_=qv[:, 2 * m : 2 * m + 2, :, :])
        nc.gpsimd.dma_start(out=kb, in_=kv[:, 2 * m : 2 * m + 2, :, :])
        nc.gpsimd.dma_start(out=vb[:, :, :, 0:128], in_=vv[:, 2 * m : 2 * m + 2, :, :])
        nc.vector.memset(vb[:, :, :, 128:129], 1.0)
        return qb, kb, vb

    with nc.named_scope("load0"):
        blk_tiles = load_block(0)

    # ---- W stream: SWDGE DMA-cast chunk -> PE transpose pairs -> DVE evac ----
    wstate = {"chunks": 0}

    def wprep_emitter():
        for nt in range(16):
            wn_c = wst.tile([128, C], BF16, tag="wnc")
            nc.gpsimd.dma_start(out=wn_c, in_=w_d[nt * 128 : (nt + 1) * 128, :])
            yield
            for kk in range(0, 16, 2):
                # transpose-mode: the identity stays resident as the
                # stationary, so these cost no per-op LDWEIGHTS
                tp = ptr.tile([128, 256], BF16, tag="tr", name="tp")
                nc.tensor.transpose(
                    tp[:, 0:128], wn_c[:, kk * 128 : (kk + 1) * 128], id_b
                )
                yield
                nc.tensor.transpose(
                    tp[:, 128:256], wn_c[:, (kk + 1) * 128 : (kk + 2) * 128], id_b
                )
                nc.vector.tensor_copy(
                    wt_sb[:, kk : kk + 2, nt * 128 : (nt + 1) * 128],
                    tp.rearrange("p (a b) -> p a b", a=2),
                )
                yield
            wstate["chunks"] = nt + 1

    # ---- projection: out[128 tok, 2048] = V @ W^T + b, K=2048 accumulated ----
    def proj_emitter(m, vt):
        ot = outp.tile([128, C], F32, tag="ot")
        for half in range(2):
            # a proj matmul must not be EMITTED before the wprep writes it
            # reads exist in the program, else Tile cannot order them
            while wstate["chunks"] < 8 * (half + 1) and wq:
                _pull(wq, 17)
            ps = [
                pmm3.tile([128, 512], F32, tag="mm3", name=f"ps{n}")
                for n in range(2)
            ]
            for kk in range(16):
                for n in range(2):
                    nn = half * 2 + n
                    nc.tensor.matmul(
                        ps[n],
                        vt[:, kk, :],
                        wt_sb[:, kk, nn * 512 : (nn + 1) * 512],
                        start=(kk == 0), stop=(kk == 15),
                    )
                    yield
            for n in range(2):
                nn = half * 2 + n
                nc.vector.tensor_add(
                    ot[:, nn * 512 : (nn + 1) * 512],
                    ps[n],
                    bias_bc[:, nn * 512 : (nn + 1) * 512],
                )
                yield
        if m == NBLK - 1:
            for nn in range(4):
                nc.sync.dma_start(
                    out=o_d[m * 128 : (m + 1) * 128, nn * 512 : (nn + 1) * 512],
                    in_=ot[:, nn * 512 : (nn + 1) * 512],
                )
        else:
            nc.sync.dma_start(out=o_d[m * 128 : (m + 1) * 128, :], in_=ot)

    # Two drain queues.  wprep is paced per block (its transposes wait on the
    # W DMA stream — drained too fast they head-block the PE queue); the
    # projection FIFO gets the rest of the budget.
    wq = [wprep_emitter()]
    pq = []
    wbudget = [0]

    def _pull(queue, k):
        while queue and k > 0:
            gen = queue[0]
            try:
                while k > 0:
                    next(gen)
                    k -= 1
            except StopIteration:
                queue.pop(0)
        return k

    def drain(k):
        if wbudget[0] > 0 and wq:
            take = min(k, wbudget[0])
            left = _pull(wq, take)
            wbudget[0] -= take - left
            k = k - (take - left)
        if k > 0:
            _pull(pq, k)

    def drain_all():
        _pull(wq, 1 << 30)
        _pull(pq, 1 << 30)

    # ---- per-(block, unit) software pipeline ----
    # q/k transpose pairs for unit u are emitted during unit u-1 (crossing
    # block seams) so the PE never sees a transpose burst.
    blk_state = {}

    def setup_block(m):
        qb, kb, vb = blk_state[m]["qkv"]
        blk_state[m]["qkt"] = qktp.tile(
            [128, 2, G, 256], BF16, tag="qkt", name=f"qkt{m}"
        )
        blk_state[m]["vt"] = vtp.tile([128, H, 128], BF16, tag="vt", name=f"vt{m}")

    def emit_pairs(m, u):
        # transpose q/k for the 4 groups of unit u of block m
        qb, kb, vb = blk_state[m]["qkv"]
        qkt = blk_state[m]["qkt"]
        bb, g0 = u // 2, (u % 2) * NU
        for j in range(4):
            drain(1)
            g = g0 + j
            trp = ptr.tile([128, 256], BF16, tag="tr", name="trp")
            nc.tensor.transpose(trp[:, 0:128], qb[:, bb, g, :], id_b)
            nc.tensor.transpose(trp[:, 128:256], kb[:, bb, g, :], id_b)
            nc.vector.tensor_copy(qkt[:, bb, g, :], trp)

    blk_state[0] = {"qkv": blk_tiles}
    setup_block(0)
    emit_pairs(0, 0)

    for m in range(NBLK):
        qb, kb, vb = blk_state[m]["qkv"]
        qkt = blk_state[m]["qkt"]
        vt = blk_state[m]["vt"]
        # wprep pacing: ~4 chunks' worth of yields per early block
        wbudget[0] = 70 if m < 4 else 0
        with nc.named_scope(f"attn{m}"):
            if m + 1 < NBLK:
                with nc.named_scope(f"load{m + 1}"):
                    blk_state[m + 1] = {"qkv": load_block(m + 1)}
            for u in range(NU):
                bb, g0 = u // 2, (u % 2) * NU
                ru = u % 2  # ring half
                # MM1: scores^T for the unit's 4 groups into one psum bank
                sc = pscp.tile([128, 4, 128], F32, tag="sc")
                for j in range(4):
                    drain(1)
                    nc.tensor.matmul(
                        sc[:, j, :],
                        qkt[:, bb, g0 + j, 128:256],
                        qkt[:, bb, g0 + j, 0:128],
                        start=True, stop=True,
                    )
                # batched exp over the 4 groups' diagonal blocks
                for lo, hi in ((0, 64), (64, 128)):
                    nc.scalar.activation(
                        exp_ring[lo:hi, 4 * ru : 4 * ru + 4, lo:hi],
                        sc[lo:hi, :, lo:hi],
                        mybir.ActivationFunctionType.Exp, scale=SCALE,
                    )
                # emit next unit's q/k transposes here so they overlap this
                # unit's exp/MM2 chain
                if u + 1 < NU:
                    emit_pairs(m, u + 1)
                elif m + 1 < NBLK:
                    setup_block(m + 1)
                    emit_pairs(m + 1, 0)
                tok = bb * 64
                for t in range(2):  # pairs of groups
                    drain(1)
                    # MM2: U = expT^T @ [v|1] -> token-major [l, 129] with
                    # the softmax denominator in column 128
                    uu = putp.tile([128, 2, 132], F32, tag="ut")
                    for jj in range(2):
                        j = 2 * t + jj
                        nc.tensor.matmul(
                            uu[:, jj, 0:129],
                            exp_ring[:, 4 * ru + j, :],
                            vb[:, bb, g0 + j, 0:129],
                            start=True, stop=True,
                        )
                        drain(2)
                    r2 = v2p.tile([128, 2], F32, tag="r2")
                    nc.vector.reciprocal(r2, uu[:, :, 128:129])
                    trv = ptr.tile([128, 256], BF16, tag="tr", name="trv")
                    for jj in range(2):
                        j = 2 * t + jj
                        v2 = v2p.tile([128, 128], BF16, tag="v2")
                        # per-partition 1/rowsum scale on the scalar engine
                        nc.scalar.mul(v2, uu[:, jj, 0:128], r2[:, jj : jj + 1])
                        nc.tensor.transpose(
                            trv[:, jj * 128 : (jj + 1) * 128], v2, id_b
                        )
                        drain(1)
                    # evacuate both V^T tiles into vt on the scalar engine
                    h0 = 2 * (g0 + 2 * t)
                    nc.scalar.copy(
                        vt[:, h0 : h0 + 4, tok : tok + 64],
                        trv.rearrange("p (a b) -> p a b", a=4),
                    )
        pq.append(proj_emitter(m, vt))
        if m == NBLK - 1:
            drain_all()


def build():
    import contextlib

    nc = bacc.Bacc("TRN2", target_bir_lowering=False, debug=False)
    q_d = nc.dram_tensor("queries", [BPC, H, L, E], F32, kind="ExternalInput").ap()
    k_d = nc.dram_tensor("keys", [BPC, H, L, E], F32, kind="ExternalInput").ap()
    v_d = nc.dram_tensor("values", [BPC, H, L, E], F32, kind="ExternalInput").ap()
    w_d = nc.dram_tensor("W", [C, C], F32, kind="ExternalInput").ap()
    b_d = nc.dram_tensor("b", [C], F32, kind="ExternalInput").ap()
    o_d = nc.dram_tensor("out", [BPC * L, C], F32, kind="ExternalOutput").ap()

    with tile.TileContext(nc) as tc:
        with contextlib.ExitStack() as ctx:
            emit(ctx, nc, tc, q_d, k_d, v_d, w_d, b_d, o_d)
    nc.compile()
    return nc


_NC_CACHE = {}


def get_nc(*a):
    if "nc" not in _NC_CACHE:
        _NC_CACHE["nc"] = build()
    return _NC_CACHE["nc"]


def make_in_maps(queries, keys, values, W, b):
    queries = np.ascontiguousarray(np.asarray(queries, dtype=np.float32))
    keys = np.ascontiguousarray(np.asarray(keys, dtype=np.float32))
    values = np.ascontiguousarray(np.asarray(values, dtype=np.float32))
    W = np.ascontiguousarray(np.asarray(W, dtype=np.float32))
    b = np.ascontiguousarray(np.asarray(b, dtype=np.float32))
    in_maps = []
    for i in range(N_CORES):
        s = slice(i * BPC, (i + 1) * BPC)
        in_maps.append(
            {
                "queries": queries[s],
                "keys": keys[s],
                "values": values[s],
                "W": W,
                "b": b,
            }
        )
    return in_maps


def kernel(queries, keys, values, W, b, **run_kwargs):
    nc = get_nc()
    in_maps = make_in_maps(queries, keys, values, W, b)
    res = run_bass_kernel_spmd(nc, in_maps, core_ids=list(range(N_CORES)), **run_kwargs)
    out = np.concatenate([res.results[i]["out"] for i in range(N_CORES)], axis=0)
    return out.reshape(B, L, C)
